# revision 49
# baseline (speedup 1.0000x reference)
"""Trainium2 Bass kernel for nn_BasicBlock (dense transformer block), v2.

Sharding: data-parallel over batch — B=8 batch elements, one per NeuronCore,
zero collectives.

v2 strategy: fp8-e4m3 matmuls with DoubleRow perf mode (2 k-tiles per PE
instruction) for all large GEMMs. Precision is protected by:
  - per-tensor power-of-2 prescales for the known input distribution
    (weights x64, q/k/v/concat x4)
  - FFN1/FFN2 in "aw" split form: W = W_hi + W_lo (host-precomputed fp8
    pair), activations a = a_hi + a_lo (device-split fp8 pair); products
    hi*hi + hi*lo + lo*hi recover ~bf16 accuracy at 0.75x fp32r PE cost
  - attention in plain fp8; causal mask = accumulating -240*28*L (fp8
    matmul) into scores PSUM so exp() underflows to exactly 0 in fp8;
    softmax denominator rides in row 65 of the BMM2 PSUM (ones column of
    vaug); token 0 (fully masked) yields 0*inf=NaN, overwritten by a
    memset on column 0 of concatT
  - folded constants: W1' = g1 (x) W1, b1' = b1 + beta1 @ W1, residual
    h1gb = g1*h1n + (beta1 + b2), xpb = x + bo (host)
"""

import numpy as np
import concourse.bass as bass
import concourse.tile as tile
from concourse import bacc, mybir
from concourse.bass_utils import run_bass_kernel_spmd

F32 = mybir.dt.float32
F32R = mybir.dt.float32r
BF16 = mybir.dt.bfloat16
F8 = mybir.dt.float8e4
U8 = mybir.dt.uint8
AF = mybir.ActivationFunctionType
OP = mybir.AluOpType
DR = mybir.MatmulPerfMode.DoubleRow

B, S, D, H, DK, DFF = 8, 1024, 1024, 16, 64, 4096
P = 128
DC = D // P       # 8 chunks of d_model
FC = DFF // P     # 32 chunks of d_ff
SC = S // P       # 8 chunks of sequence
EPS = 1e-5
SW = 64.0         # weight prescale
SQ = 4.0          # q/k/v/concat prescale
EXPSC = 0.125 / (SQ * SQ)   # exp reads scores*(SQ*SQ)


def _build():
    nc = bacc.Bacc("TRN2", target_bir_lowering=False, debug=False, num_devices=B)

    def dram(name, shape, dt):
        return nc.dram_tensor(name, shape, dt, kind="ExternalInput").ap()

    x8T_d = dram("x8T", [D, S], U8)
    xpb_d = dram("xpb", [S, D], F32)
    wq8_d = dram("wq8", [D, D], U8)
    wk8_d = dram("wk8", [D, D], U8)
    wv8_d = dram("wv8", [D, D], U8)
    wo8_d = dram("wo8", [D, D], U8)
    w1hl8_d = dram("w1hl8", [FC, P, 2, DC, P], U8)   # [c, p, hl, k, fcol]
    w2hl8_d = dram("w2hl8", [2, FC // 2, P, 2, 2, 512], U8)  # [nh,kp,p,hl,a,d]
    bq4_d = dram("bq4", [D], F32)
    bk4_d = dram("bk4", [D], F32)
    bv4_d = dram("bv4", [D], F32)
    b1p_d = dram("b1p", [DFF], F32)
    g1_d = dram("g1v", [D], F32)
    bb2_d = dram("bb2", [D], F32)
    g3_d = dram("g3v", [D], F32)
    b3_d = dram("b3v", [D], F32)
    id8_d = dram("id8", [P, P], U8)
    negi8_d = dram("negi8", [P, P], U8)
    l28_d = dram("l28", [P, P], U8)
    ones_d = dram("ones", [P, P], F32)
    out_d = nc.dram_tensor("out", [S, D], F32, kind="ExternalOutput").ap()

    def bcast_ap(dram_ap, n):
        return bass.AP(tensor=dram_ap.tensor, offset=dram_ap.offset,
                       ap=[[0, P], [1, n]])

    with tile.TileContext(nc) as tc:
      with tc.tile_pool(name="singles", bufs=1) as singles, \
           tc.tile_pool(name="sm", bufs=4) as sm:
        id8_sb = singles.tile([P, P], F8)
        negi8_sb = singles.tile([P, P], F8)
        l28_sb = singles.tile([P, P], F8)
        eps_t = singles.tile([P, 1], F32)
        ones_sb = singles.tile([1, DK], F32R)
        bq4_sb = singles.tile([P, DC], F32)
        bk4_sb = singles.tile([P, DC], F32)
        b1p_sb = singles.tile([P, FC], F32)
        g1b = singles.tile([P, D], F32)
        bb2b = singles.tile([P, D], F32)
        g3b = singles.tile([P, D], F32)
        b3b = singles.tile([P, D], F32)
        nc.vector.memset(eps_t[:], EPS)
        nc.sync.dma_start(id8_sb[:], id8_d.bitcast(F8))
        nc.sync.dma_start(negi8_sb[:], negi8_d.bitcast(F8))
        nc.sync.dma_start(l28_sb[:], l28_d.bitcast(F8))
        nc.sync.dma_start(ones_sb[:], ones_d[0:1, 0:DK].bitcast(F32R))
        nc.sync.dma_start(bq4_sb[:], bq4_d.rearrange("(c p) -> p c", p=P))
        nc.sync.dma_start(bk4_sb[:], bk4_d.rearrange("(c p) -> p c", p=P))
        nc.sync.dma_start(b1p_sb[:], b1p_d.rearrange("(c p) -> p c", p=P))
        nc.sync.dma_start(g1b[:], bcast_ap(g1_d, D))
        nc.sync.dma_start(bb2b[:], bcast_ap(bb2_d, D))
        nc.sync.dma_start(g3b[:], bcast_ap(g3_d, D))
        nc.sync.dma_start(b3b[:], bcast_ap(b3_d, D))

        def ln_scalars(stats_ap):
            """bn-aggregated stats -> (rstd [P,1], nb [P,1])."""
            mv = sm.tile([P, 2], F32, tag="mv", name="mv")
            rs = sm.tile([P, 1], F32, tag="rs", name="rs")
            nb = sm.tile([P, 1], F32, tag="nb", name="nb")
            nc.vector.bn_aggr(mv[:], stats_ap)
            nc.scalar.activation(rs[:], mv[:, 1:2], AF.Sqrt,
                                 bias=eps_t[:], scale=1.0)
            nc.vector.reciprocal(rs[:], rs[:])
            nc.vector.tensor_scalar(nb[:], mv[:, 0:1], rs[:], -1.0,
                                    op0=OP.mult, op1=OP.mult)
            return rs, nb

        # ================= residual stream (lives through phase E) ========
        with tc.tile_pool(name="resid", bufs=1) as resid:
            xpb_sb = resid.tile([P, SC, D], F32)     # x + bo; later h1gb
            h1pre_sb = resid.tile([P, SC, D], F32)   # attn residual; later z2
            h1p_cm = tc.tile_pool(name="h1p", bufs=1)
            h1p = h1p_cm.__enter__()
            h1hiT = h1p.tile([P, DC, S], F8)
            h1loT = h1p.tile([P, DC, S], F8)
            cw_cm = tc.tile_pool(name="cw", bufs=1)
            cw = cw_cm.__enter__()
            concat8_sb = cw.tile([P, DC, S], F8)
            wo8_sb = cw.tile([P, DC, D], F8)
            # token-0 column of concatT: fully-masked attention yields
            # 0*inf = NaN in the normalization; reference zero-pads it.
            # The per-head muls write cols [1:S] only; col 0 stays 0.
            nc.vector.memset(concat8_sb[:, :, 0:1], 0.0)

            # ============ phases A+B: QKV projections + attention =========
            with tc.tile_pool(name="qkv", bufs=1) as qkv:
                # qT8 carries a zero k-tile slab per chunk (DoubleRow pair);
                # kT8 is plain with one zero pad chunk at the end — the
                # k-side second k-tile is real data from chunk ch+1, nulled
                # by the q-side zero slab (0 * data = 0).
                qT8 = qkv.tile([P, DC, 2, S], F8)
                kT8 = qkv.tile([P, DC + 1, S], F8)
                vaug8 = qkv.tile([P, SC, H, DK + 1], F8)

                nc.scalar.memzero(qT8[:, :, 1, :])
                nc.scalar.memzero(kT8[:, DC, :])
                nc.vector.memset(vaug8[:, :, :, DK:DK + 1], 1.0)

                with tc.tile_pool(name="wqk", bufs=1) as wqk, \
                     tc.tile_pool(name="psA", bufs=4, space="PSUM") as psA:
                    x8T_sb = wqk.tile([P, DC, S], F8)
                    bv4b = wqk.tile([P, D], F32)
                    nc.sync.dma_start(bv4b[:], bcast_ap(bv4_d, D))
                    wq8_sb = wqk.tile([P, DC, D], F8)
                    wk8_sb = wqk.tile([P, DC, D], F8)
                    wv8_sb = wqk.tile([P, DC, D], F8)
                    nc.sync.dma_start(
                        x8T_sb[:], x8T_d.rearrange("(k p) s -> p k s", p=P)
                        .bitcast(F8))
                    for wsb, wd in ((wv8_sb, wv8_d), (wq8_sb, wq8_d),
                                    (wk8_sb, wk8_d)):
                        nc.sync.dma_start(
                            wsb[:], wd.rearrange("(k p) d -> p k d", p=P)
                            .bitcast(F8))
                    nc.sync.dma_start(
                        wo8_sb[:], wo8_d.rearrange("(k p) d -> p k d", p=P)
                        .bitcast(F8))
                    for i in range(2):
                        nc.sync.dma_start(
                            xpb_sb[:, 4 * i:4 * (i + 1), :],
                            xpb_d[4 * P * i:4 * P * (i + 1), :]
                            .rearrange("(a p) d -> p a d", p=P))

                    # V projection first (its epilogue runs on DVE, so the
                    # last psA tile frees quickly before attention), then Q/K
                    for m in range(SC):
                        ps = psA.tile([P, S], F32, tag="acc", name="acc")
                        for n in range(2):
                            cols = slice(512 * n, 512 * (n + 1))
                            for t in range(4):
                                nc.tensor.matmul(
                                    ps[:, cols],
                                    x8T_sb[:, 2 * t:2 * t + 2,
                                           P * m:P * (m + 1)],
                                    wv8_sb[:, 2 * t:2 * t + 2, cols],
                                    start=(t == 0), stop=(t == 3),
                                    perf_mode=DR, skip_group_check=True)
                        nc.vector.scalar_tensor_tensor(
                            vaug8[:, m, :, 0:DK],
                            ps[:].rearrange("p (h d) -> p h d", d=DK),
                            1.0 / 16.0,
                            bv4b[:].rearrange("p (h d) -> p h d", d=DK),
                            op0=OP.mult, op1=OP.add)
                    for (wsb, is_q, bias) in ((wq8_sb, True, bq4_sb),
                                              (wk8_sb, False, bk4_sb)):
                        for c in range(DC):
                            ps = psA.tile([P, S], F32, tag="acc", name="acc")
                            for n in range(2):
                                cols = slice(512 * n, 512 * (n + 1))
                                for t in range(4):
                                    nc.tensor.matmul(
                                        ps[:, cols],
                                        wsb[:, 2 * t:2 * t + 2,
                                            P * c:P * (c + 1)],
                                        x8T_sb[:, 2 * t:2 * t + 2, cols],
                                        start=(t == 0), stop=(t == 3),
                                        perf_mode=DR, skip_group_check=True)
                            if is_q:
                                nc.scalar.activation(
                                    qT8[:, c, 0, :], ps[:], AF.Identity,
                                    bias=bias[:, c:c + 1], scale=1.0 / 16.0)
                            else:
                                nc.vector.tensor_scalar(
                                    kT8[:, c, :], ps[:], 1.0 / 16.0,
                                    bias[:, c:c + 1], op0=OP.mult,
                                    op1=OP.add)

                # -------- attention, head-pipelined --------
                with tc.tile_pool(name="expp", bufs=3) as expp, \
                     tc.tile_pool(name="dsm", bufs=2) as dsm, \
                     tc.tile_pool(name="psS", bufs=2, space="PSUM") as psS, \
                     tc.tile_pool(name="psC", bufs=1, space="PSUM") as psC, \
                     tc.tile_pool(name="psB", bufs=1, space="PSUM") as psB:
                    expT_tiles = {}

                    def emit_scores(h):
                        ch, off = h // 2, DK * (h % 2)
                        expT = expp.tile([P, SC, S], F8, tag="expT",
                                         name=f"expT{h}")
                        expT_tiles[h] = expT
                        for j in range(SC):
                            lo = P * j
                            ps = psS.tile([P, S], F32, tag="sc", name="sc")
                            ranges = ([(lo, 512), (512, 1024)] if lo < 512
                                      else [(lo, 1024)])
                            for (c0, c1) in ranges:
                                nc.tensor.matmul(
                                    ps[:, c0:c1],
                                    kT8[off:off + DK, ch:ch + 2,
                                        P * j:P * (j + 1)],
                                    qT8[off:off + DK, ch, 0:2, c0:c1],
                                    start=True, stop=False,
                                    perf_mode=DR, skip_group_check=True)
                            # causal mask: accumulate -6720*L on diag block
                            nc.tensor.matmul(
                                ps[:, lo:lo + P], negi8_sb[:], l28_sb[:],
                                start=False, stop=True,
                                skip_group_check=True)
                            nc.scalar.activation(
                                expT[:, j, lo:S], ps[:, lo:S],
                                AF.Exp, bias=0.0, scale=EXPSC)

                    def emit_bmm2(h):
                        ch, off = h // 2, DK * (h % 2)
                        expT = expT_tiles.pop(h)
                        psc = psC.tile([DK + 1, S], F32, tag="ctx",
                                       name="ctx")
                        evs = []
                        for t in range(4):
                            evs.append(("pair", t, P * (2 * t + 1), S))
                            evs.append(("single", 2 * t, 256 * t,
                                        256 * t + P))
                        for n in range(2):
                            nlo, nhi = 512 * n, 512 * (n + 1)
                            todo = []
                            for kind, t, v0, v1 in evs:
                                a, b = max(v0, nlo), min(v1, nhi)
                                if a < b:
                                    todo.append((kind, t, a, b))
                            for idx, (kind, t, a, b) in enumerate(todo):
                                st = (idx == 0)
                                sp = (idx == len(todo) - 1)
                                if kind == "pair":
                                    nc.tensor.matmul(
                                        psc[:, a:b],
                                        vaug8[:, 2 * t:2 * t + 2, h, :],
                                        expT[:, 2 * t:2 * t + 2, a:b],
                                        start=st, stop=sp,
                                        perf_mode=DR, skip_group_check=True)
                                else:
                                    nc.tensor.matmul(
                                        psc[:, a:b],
                                        vaug8[:, t, h, :],
                                        expT[:, t, a:b],
                                        start=st, stop=sp,
                                        skip_group_check=True)
                        den1 = dsm.tile([1, S], F32R, tag="den1",
                                        name="den1")
                        rec64 = dsm.tile([DK, S], F32, tag="rec64",
                                         name="rec64")
                        tmp = dsm.tile([DK, S], F8, tag="tmp", name="tmp")
                        nc.vector.tensor_scalar_add(den1[:],
                                                    psc[DK:DK + 1, :], 1e-6)
                        rps = psB.tile([DK, S], F32, tag="bc", name="bc")
                        for n in range(2):
                            cols = slice(512 * n, 512 * (n + 1))
                            nc.tensor.matmul(
                                rps[:, cols],
                                ones_sb[0:1, :],
                                den1[0:1, cols],
                                start=True, stop=True,
                                skip_group_check=True)
                        nc.vector.reciprocal(rec64[:], rps[:])
                        if off == 0:
                            nc.vector.tensor_mul(concat8_sb[0:DK, ch, 1:S],
                                                 psc[0:DK, 1:S],
                                                 rec64[:, 1:S])
                        else:
                            nc.vector.tensor_mul(tmp[:, 1:S], psc[0:DK, 1:S],
                                                 rec64[:, 1:S])
                            nc.gpsimd.dma_start(concat8_sb[DK:P, ch, 1:S],
                                                tmp[:, 1:S])

                    horder = []
                    for hp in range(H // 2):
                        horder += [2 * hp + 1, 2 * hp]
                    for i, h in enumerate(horder):
                        emit_scores(h)
                        if i >= 2:
                            emit_bmm2(horder[i - 2])
                    emit_bmm2(horder[H - 2])
                    emit_bmm2(horder[H - 1])

            # ============ phase C: out-proj + LN1 + splits + transposes ===
            if True:
                with tc.tile_pool(name="cpool", bufs=1) as cpool, \
                     tc.tile_pool(name="psA2", bufs=2, space="PSUM") as psA2, \
                     tc.tile_pool(name="psT", bufs=4, space="PSUM") as psT:
                    h1n32 = cpool.tile([P, SC, D], F32)
                    h1hi8 = cpool.tile([P, SC, D], F8)
                    h1lo8 = cpool.tile([P, SC, D], F8)
                    for m in range(SC):
                        ps = psA2.tile([P, S], F32, tag="op", name="op")
                        for n in range(2):
                            cols = slice(512 * n, 512 * (n + 1))
                            for t in range(4):
                                nc.tensor.matmul(
                                    ps[:, cols],
                                    concat8_sb[:, 2 * t:2 * t + 2,
                                               P * m:P * (m + 1)],
                                    wo8_sb[:, 2 * t:2 * t + 2, cols],
                                    start=(t == 0), stop=(t == 3),
                                    perf_mode=DR, skip_group_check=True)
                        nc.vector.scalar_tensor_tensor(
                            h1pre_sb[:, m, :], ps[:], 1.0 / 256.0,
                            xpb_sb[:, m, :], op0=OP.mult, op1=OP.add)
                        stm = sm.tile([P, 2, 6], F32, tag="st", name="st")
                        for n in range(2):
                            nc.vector.bn_stats(
                                stm[:, n, :],
                                h1pre_sb[:, m, 512 * n:512 * (n + 1)])
                        rs, nb = ln_scalars(stm[:])
                        # hi8 straight from h1pre on ACT (parallel with the
                        # DVE h1n32 pass), lo8 from the difference
                        nc.scalar.activation(h1hi8[:, m, :],
                                             h1pre_sb[:, m, :],
                                             AF.Identity, bias=nb[:],
                                             scale=rs[:])
                        nc.vector.tensor_scalar(
                            h1n32[:, m, :], h1pre_sb[:, m, :], rs[:], nb[:],
                            op0=OP.mult, op1=OP.add)
                        nc.vector.scalar_tensor_tensor(
                            h1lo8[:, m, :], h1n32[:, m, :], 1.0,
                            h1hi8[:, m, :], op0=OP.mult, op1=OP.subtract)
                        # h1gb = g1*h1n + (beta1+b2), stored over xpb
                        nc.gpsimd.tensor_mul(xpb_sb[:, m, :],
                                             h1n32[:, m, :], g1b[:])
                        nc.gpsimd.tensor_add(xpb_sb[:, m, :],
                                             xpb_sb[:, m, :], bb2b[:])
                    # transposes of hi/lo into feature-major (second loop so
                    # the PE is not stalled behind each m's LN/split chain);
                    # one 8-wide PSUM batch + a single copy per (m, tensor),
                    # copies split ACT/DVE
                    for m in range(SC):
                        for src, dstT, on_act in ((h1hi8, h1hiT, True),
                                                  (h1lo8, h1loT, False)):
                            # fp8 PE transpose requires output element step 2
                            pt = psT.tile([P, DC, P, 2], F8, tag="pt",
                                          name="pt")
                            for i in range(DC):
                                nc.tensor.matmul(
                                    pt[:, i, :, 0],
                                    src[:, m, P * i:P * (i + 1)],
                                    id8_sb[:], is_transpose=True,
                                    start=True, stop=True,
                                    skip_group_check=True)
                            dst = dstT[:, :, P * m:P * (m + 1)]
                            nc.scalar.copy(dst, pt[:, :, :, 0])

                cw_cm.__exit__(None, None, None)

                # ============ phase D: FFN1 (aw split) ====================
                with tc.tile_pool(name="ftp", bufs=1) as ftp, \
                     tc.tile_pool(name="w2s", bufs=4) as w2s:
                    fThi = ftp.tile([P, FC, S], F8)
                    fTlo = ftp.tile([P, FC, S], F8)

                    w2_order = [(nh, kp) for nh in range(2)
                                for kp in range(FC // 2)]
                    w2_tiles = {}

                    def load_w2(i):
                        if i >= len(w2_order):
                            return
                        nh_, kp_ = w2_order[i]
                        tl = w2s.tile([P, 2, 2, 512], F8, tag="w2",
                                      name=f"w2{nh_}_{kp_}")
                        nc.sync.dma_start(tl[:],
                                          w2hl8_d[nh_, kp_].bitcast(F8))
                        w2_tiles[(nh_, kp_)] = tl

                    with tc.tile_pool(name="w1s", bufs=4) as w1s, \
                         tc.tile_pool(name="f32s", bufs=2) as f32s, \
                         tc.tile_pool(name="psF1", bufs=4,
                                      space="PSUM") as psF1:
                      for c in range(FC):
                        if c == FC - 3:
                            for i in range(3):
                                load_w2(i)
                        w1_t = w1s.tile([P, 2, DC, P], F8, tag="w1",
                                        name=f"w1{c}")
                        nc.sync.dma_start(w1_t[:], w1hl8_d[c].bitcast(F8))
                        ps = psF1.tile([P, S], F32, tag="f1", name="f1")
                        for n in range(2):
                            cols = slice(512 * n, 512 * (n + 1))
                            for t in range(4):
                                kt = slice(2 * t, 2 * t + 2)
                                nc.tensor.matmul(
                                    ps[:, cols], w1_t[:, 0, kt, :],
                                    h1hiT[:, kt, cols],
                                    start=(t == 0), stop=False,
                                    perf_mode=DR, skip_group_check=True)
                                nc.tensor.matmul(
                                    ps[:, cols], w1_t[:, 1, kt, :],
                                    h1hiT[:, kt, cols],
                                    start=False, stop=False,
                                    perf_mode=DR, skip_group_check=True)
                                nc.tensor.matmul(
                                    ps[:, cols], w1_t[:, 0, kt, :],
                                    h1loT[:, kt, cols],
                                    start=False, stop=(t == 3),
                                    perf_mode=DR, skip_group_check=True)
                        pre32 = f32s.tile([P, S], F32, tag="pre",
                                          name=f"pre{c}")
                        nc.scalar.activation(pre32[:], ps[:], AF.Identity,
                                             bias=b1p_sb[:, c:c + 1],
                                             scale=1.0 / SW)
                        nc.scalar.activation(fThi[:, c, :], pre32[:],
                                             AF.Relu, bias=0.0, scale=1.0)
                        nc.vector.scalar_tensor_tensor(
                            fTlo[:, c, :], pre32[:], 0.0, fThi[:, c, :],
                            op0=OP.max, op1=OP.subtract)

                    # ============ phase E: FFN2 (aw) + LN2 ================
                    stF = [sm.tile([P, 2, 6], F32, tag=f"stF{m}",
                                   name=f"stF{m}") for m in range(SC)]
                    with tc.tile_pool(name="psF2", bufs=1,
                                      space="PSUM") as psF2, \
                         tc.tile_pool(name="ostg", bufs=2) as ostg:
                        for nh in range(2):
                            ncols = slice(512 * nh, 512 * (nh + 1))
                            zps = [psF2.tile([P, 512], F32, tag=f"z{m}",
                                             name=f"z{nh}_{m}")
                                   for m in range(SC)]
                            for kp in range(FC // 2):
                                w2_t = w2_tiles.pop((nh, kp))
                                load_w2(nh * (FC // 2) + kp + 3)
                                for m in range(SC):
                                    kt = slice(2 * kp, 2 * kp + 2)
                                    mcols = slice(P * m, P * (m + 1))
                                    nc.tensor.matmul(
                                        zps[m][:], fThi[:, kt, mcols],
                                        w2_t[:, 0, :, :], start=(kp == 0),
                                        stop=False, perf_mode=DR,
                                        skip_group_check=True)
                                    nc.tensor.matmul(
                                        zps[m][:], fThi[:, kt, mcols],
                                        w2_t[:, 1, :, :], start=False,
                                        stop=False,
                                        perf_mode=DR, skip_group_check=True)
                                    nc.tensor.matmul(
                                        zps[m][:], fTlo[:, kt, mcols],
                                        w2_t[:, 0, :, :], start=False,
                                        stop=(kp == FC // 2 - 1),
                                        perf_mode=DR, skip_group_check=True)
                            for m in range(SC):
                                nc.vector.scalar_tensor_tensor(
                                    h1pre_sb[:, m, ncols], zps[m][:],
                                    1.0 / SW, xpb_sb[:, m, ncols],
                                    op0=OP.mult, op1=OP.add)
                                nc.vector.bn_stats(stF[m][:, nh, :],
                                                   h1pre_sb[:, m, ncols])
                        for m in range(SC):
                            rs, nb = ln_scalars(stF[m][:])
                            o_t = ostg.tile([P, D], F32, tag="o",
                                            name=f"o{m}")
                            nc.vector.tensor_scalar(
                                o_t[:], h1pre_sb[:, m, :], rs[:], nb[:],
                                op0=OP.mult, op1=OP.add)
                            nc.gpsimd.tensor_mul(o_t[:], o_t[:], g3b[:])
                            nc.vector.tensor_add(o_t[:], o_t[:], b3b[:])
                            nc.sync.dma_start(out_d[P * m:P * (m + 1), :],
                                              o_t[:])
                h1p_cm.__exit__(None, None, None)

    nc.compile()
    return nc


_cached = None


def _get_prog():
    global _cached
    if _cached is None:
        _cached = _build()
    return _cached


def _q8(a):
    import ml_dtypes
    return np.asarray(a, ml_dtypes.float8_e4m3)


def _prep_common(inputs):
    f = {k: np.asarray(inputs[k], np.float32) for k in inputs}
    E = lambda a: _q8(a).view(np.uint8)
    g1 = f["g1"]
    w1p = g1[:, None] * f["W1"]
    b1p = f["b1"] + f["beta1"] @ f["W1"]
    w1s = w1p * SW
    w1h = _q8(w1s)
    w1l = _q8(w1s - w1h.astype(np.float32))
    w2s = f["W2"] * SW
    w2h = _q8(w2s)
    w2l = _q8(w2s - w2h.astype(np.float32))

    def r1(w8):  # [D, DFF] -> [FC, P, DC, P]
        return w8.view(np.uint8).reshape(DC, P, FC, P).transpose(2, 1, 0, 3)

    def r2(w8):  # [DFF, D] -> [2, FC//2, P, 2, 512]
        return (w8.view(np.uint8).reshape(FC // 2, 2, P, 2, 512)
                .transpose(3, 0, 2, 1, 4))

    common = {
        "wq8": E(f["Wq"] * SW), "wk8": E(f["Wk"] * SW),
        "wv8": E(f["Wv"] * SW), "wo8": E(f["Wo"] * SW),
        "w1hl8": np.ascontiguousarray(
            np.stack([r1(w1h), r1(w1l)], axis=2)),
        "w2hl8": np.ascontiguousarray(
            np.stack([r2(w2h), r2(w2l)], axis=3)),
        "bq4": f["bq"] * SQ, "bk4": f["bk"] * SQ, "bv4": f["bv"] * SQ,
        "b1p": b1p, "g1v": g1, "bb2": f["beta1"] + f["b2"],
        "g3v": f["g3"], "b3v": f["beta3"],
        "id8": _q8(np.eye(P, dtype=np.float32)).view(np.uint8),
        "negi8": _q8(-240.0 * np.eye(P, dtype=np.float32)).view(np.uint8),
        "l28": _q8(28.0 * np.tril(np.ones((P, P), np.float32))).view(np.uint8),
        "ones": np.ones((P, P), np.float32),
    }
    return common, f


def kernel(**inputs):
    x = np.asarray(inputs["x"], dtype=np.float32)
    assert x.shape == (B, S, D)
    common, f = _prep_common(inputs)
    in_maps = []
    for i in range(B):
        xi = x[i]
        in_maps.append(dict(
            common,
            x8T=np.ascontiguousarray(_q8(xi.T).view(np.uint8)),
            xpb=np.ascontiguousarray(xi + f["bo"]),
        ))
    nc = _get_prog()
    res = run_bass_kernel_spmd(nc, in_maps, list(range(B)))
    return np.stack([res.results[i]["out"] for i in range(B)], axis=0)
